# revision 5
# baseline (speedup 1.0000x reference)
"""Causal self-attention (B=4, T=1024, C=1024, H=16) on 8 trn2 NeuronCores.

Sharding: core i handles batch b = i // 2 and head-group hg = i % 2
(8 heads = 512 of the 1024 channel dims). Each core computes

    qkv       = x[b] @ W_qkv[:, local]          (fp32r matmuls)
    P^T       = exp((k_h^T q_h) / 8) (causal)    (unstable softmax, bf16 P)
    y'^T      = [v_h | 1]^T @ P^T                (bf16, gives y^T + row-sums D)
    y^T       = y'^T / D                          (DMA remap + bcast + DVE)
    partial   = y^T.T @ W_proj[local, :]          (bf16)

Host sums the two head-group partials per batch and adds b_proj.

The qk projections, S^T matmuls, and V matmuls are software-pipelined per
head pair so the exp work on the scalar engine overlaps PE matmuls of the
next head pair. S^T blocks are column-restricted to the causal region and
diagonal blocks masked by a multiplicative upper-triangular mask post-exp.

Denominator normalization: the V matmul's ones-column gives row sums on
PSUM partition 64; a DMA remaps that row to SBUF partition 0 (gpsimd
partition_broadcast only reads physical partition 0), then broadcast +
reciprocal + multiply. Odd heads stage the normalized y at partition 0 and
DMA-remap into yT rows 64-127 (DVE ops never straddle partition bases).
"""

import numpy as np
from contextlib import ExitStack

import ml_dtypes

import concourse.bacc as bacc
import concourse.tile as tile
import concourse.mybir as mybir
from concourse.bass_utils import run_bass_kernel_spmd
from concourse.masks import make_upper_triangular

B, T, C, H, HD = 4, 1024, 1024, 16, 64
NCORES = 8
HPG = 8            # heads per core
DL = HPG * HD      # 512 local channel dims per core
P = 128

F32 = mybir.dt.float32
F32R = mybir.dt.float32r
BF16 = mybir.dt.bfloat16
EXP = mybir.ActivationFunctionType.Exp

MM_F32R = True     # fp32r for qkv / S^T matmuls (vs fp32, 4x slower)
PV_BF16 = True     # bf16 for P, v, y^T, wp (V-matmul + proj at full rate)

PV = BF16 if PV_BF16 else F32
MMDT = F32R if MM_F32R else F32


def _build_program():
    nc = bacc.Bacc("TRN2", target_bir_lowering=False)

    xT = nc.dram_tensor("xT", [C, T], MMDT, kind="ExternalInput").ap()
    wq = nc.dram_tensor("wq", [C, DL], MMDT, kind="ExternalInput").ap()
    wk = nc.dram_tensor("wk", [C, DL], MMDT, kind="ExternalInput").ap()
    wv = nc.dram_tensor("wv", [C, DL], MMDT, kind="ExternalInput").ap()
    wp = nc.dram_tensor("wp", [DL, C], PV, kind="ExternalInput").ap()
    bq = nc.dram_tensor("bq", [P, 4], F32, kind="ExternalInput").ap()
    bk = nc.dram_tensor("bk", [P, 4], F32, kind="ExternalInput").ap()
    bv = nc.dram_tensor("bv", [P, HPG, HD], F32, kind="ExternalInput").ap()
    outp = nc.dram_tensor("outp", [T, C], F32, kind="ExternalOutput").ap()

    with tile.TileContext(nc) as tc:
        with ExitStack() as ctx:
            consts = ctx.enter_context(tc.tile_pool(name="consts", bufs=1))
            xt_pool = ctx.enter_context(tc.tile_pool(name="xt", bufs=8))
            w_pool = ctx.enter_context(tc.tile_pool(name="w", bufs=4))
            qk_pool = ctx.enter_context(tc.tile_pool(name="qk", bufs=1))
            v_pool = ctx.enter_context(tc.tile_pool(name="v", bufs=8))
            pt_pool = ctx.enter_context(tc.tile_pool(name="pt", bufs=52))
            yt_pool = ctx.enter_context(tc.tile_pool(name="yt", bufs=1))
            d_pool = ctx.enter_context(tc.tile_pool(name="d", bufs=4))
            stg_pool = ctx.enter_context(tc.tile_pool(name="stg", bufs=3))
            out_pool = ctx.enter_context(tc.tile_pool(name="out", bufs=4))
            ps = ctx.enter_context(tc.tile_pool(name="ps", bufs=8, space="PSUM"))

            # ---- constants ----
            tri = consts.tile([P, P], PV, name="tri")  # 1 where tq >= s
            make_upper_triangular(nc, tri[:], val=1.0, diag=True)

            bq_sb = consts.tile([P, 4], F32, name="bq")
            nc.sync.dma_start(bq_sb[:], bq)
            bk_sb = consts.tile([P, 4], F32, name="bk")
            nc.sync.dma_start(bk_sb[:], bk)
            bv_sb = consts.tile([P, HPG, HD], F32, name="bv")
            nc.sync.dma_start(bv_sb[:], bv)

            wp_sb = consts.tile([P, 4, C], PV, name="wp")
            for dc in range(4):
                nc.sync.dma_start(wp_sb[:, dc, :], wp[P * dc : P * (dc + 1), :])

            xt = []
            for cc in range(8):
                t_ = xt_pool.tile([P, T], MMDT, name="xt")
                nc.sync.dma_start(t_[:], xT[P * cc : P * (cc + 1), :])
                xt.append(t_)

            # v tiles: [s=128, 8 heads x (64 dims + ones col)]
            v_sb = []
            for j in range(8):
                vt = v_pool.tile([P, HPG * (HD + 1)], PV, name="v")
                ones_cols = vt[:].rearrange("p (h e) -> p h e", e=HD + 1)[:, :, HD : HD + 1]
                nc.vector.memset(ones_cols, 1.0)
                v_sb.append(vt)

            qT_sb = qk_pool.tile([P, 4, T], MMDT, name="qT")
            kT_sb = qk_pool.tile([P, 4, T], MMDT, name="kT")
            yT_sb = yt_pool.tile([P, 4, T], PV, name="yT")

            # ---- v projection (needed by every head pair) ----
            ps_t = [ps.tile([P, 512], F32, name="ps") for _ in range(8)]
            for cc in range(8):
                wt = w_pool.tile([P, DL], MMDT, name="wv")
                nc.sync.dma_start(wt[:], wv[P * cc : P * (cc + 1), :])
                for tt in range(8):
                    nc.tensor.matmul(
                        ps_t[tt][:, :],
                        lhsT=xt[cc][:, P * tt : P * (tt + 1)],
                        rhs=wt[:],
                        start=(cc == 0),
                        stop=(cc == 7),
                    )
            for tt in range(8):
                out_ap = v_sb[tt][:].rearrange("p (h e) -> p h e", e=HD + 1)[:, :, 0:HD]
                in_ap = ps_t[tt][:].rearrange("p (h e) -> p h e", e=HD)
                nc.vector.tensor_add(out_ap, in_ap, bv_sb[:, :, :])

            def issue_qk(dt_):
                """q and k projections for head pair dt_ (128 channel dims)."""
                for w_dram, bias_sb, dest in ((wq, bq_sb, qT_sb), (wk, bk_sb, kT_sb)):
                    pst = [ps.tile([P, 512], F32, name="ps") for _ in range(2)]
                    for cc in range(8):
                        wt = w_pool.tile([P, P], MMDT, name="wqk")
                        nc.sync.dma_start(
                            wt[:],
                            w_dram[P * cc : P * (cc + 1), P * dt_ : P * (dt_ + 1)],
                        )
                        for tch in range(2):
                            nc.tensor.matmul(
                                pst[tch][:, :],
                                lhsT=wt[:],
                                rhs=xt[cc][:, 512 * tch : 512 * (tch + 1)],
                                start=(cc == 0),
                                stop=(cc == 7),
                            )
                    for tch in range(2):
                        nc.vector.tensor_scalar_add(
                            dest[:, dt_, 512 * tch : 512 * (tch + 1)],
                            pst[tch][:, :],
                            bias_sb[:, dt_ : dt_ + 1],
                        )

            def issue_st(hp):
                """S^T blocks + exp + causal mask for head pair hp."""
                pts = {}
                for c in range(2):
                    for j in range(4 * c + 4):
                        off = max(0, P * (j - 4 * c))
                        n = 512 - off
                        for half in range(2):
                            pr = 64 * half
                            pss = ps.tile([P, 512], F32, name="ps")
                            nc.tensor.matmul(
                                pss[:, :n],
                                lhsT=kT_sb[pr : pr + 64, hp, P * j : P * (j + 1)],
                                rhs=qT_sb[
                                    pr : pr + 64, hp, 512 * c + off : 512 * (c + 1)
                                ],
                                start=True,
                                stop=True,
                            )
                            pt = pt_pool.tile([P, 512], PV, name="pt")
                            nc.scalar.activation(
                                out=pt[:, off:512], in_=pss[:, :n], func=EXP, scale=0.125
                            )
                            if j >= 4 * c:
                                nc.vector.tensor_mul(
                                    pt[:, off : off + P], pt[:, off : off + P], tri[:]
                                )
                            pts[(half, c, j)] = pt
                return pts

            def issue_v(hp, pts):
                """V matmuls + denominator normalization for head pair hp."""
                for half in range(2):
                    h = 2 * hp + half
                    pr = 64 * half
                    for c in range(2):
                        jmax = 4 * c + 3
                        py = ps.tile([P, 512], F32, name="ps")
                        for j in range(jmax + 1):
                            off = max(0, P * (j - 4 * c))
                            nc.tensor.matmul(
                                py[0 : HD + 1, off:512],
                                lhsT=v_sb[j][:, (HD + 1) * h : (HD + 1) * (h + 1)],
                                rhs=pts[(half, c, j)][:, off:512],
                                start=(j == 0),
                                stop=(j == jmax),
                            )
                        d2 = d_pool.tile([P, 512], F32, name="d")
                        nc.vector.tensor_copy(d2[HD : HD + 1, :], py[HD : HD + 1, :])
                        nc.sync.dma_start(d2[0:1, :], d2[HD : HD + 1, :])
                        nc.gpsimd.partition_broadcast(d2[0:HD, :], d2[0:1, :])
                        nc.vector.reciprocal(d2[0:HD, :], d2[0:HD, :])
                        dst = yT_sb[pr : pr + HD, hp, 512 * c : 512 * (c + 1)]
                        if half == 0:
                            nc.vector.tensor_mul(dst, py[0:HD, :], d2[0:HD, :])
                        else:
                            stg = stg_pool.tile([HD, 512], PV, name="stg")
                            nc.vector.tensor_mul(stg[:], py[0:HD, :], d2[0:HD, :])
                            nc.sync.dma_start(dst, stg[:])

            # ---- pipelined qk + attention ----
            issue_qk(0)
            pts_cur = issue_st(0)
            for hp in range(4):
                if hp + 1 < 4:
                    issue_qk(hp + 1)
                    pts_next = issue_st(hp + 1)
                else:
                    pts_next = None
                issue_v(hp, pts_cur)
                pts_cur = pts_next

            # ---- output projection ----
            for tt in range(8):
                pouts = [ps.tile([P, 512], F32, name="ps") for _ in range(2)]
                for dc in range(4):
                    for cch in range(2):
                        nc.tensor.matmul(
                            pouts[cch][:, :],
                            lhsT=yT_sb[:, dc, P * tt : P * (tt + 1)],
                            rhs=wp_sb[:, dc, 512 * cch : 512 * (cch + 1)],
                            start=(dc == 0),
                            stop=(dc == 3),
                        )
                for cch in range(2):
                    ot = out_pool.tile([P, 512], F32, name="out")
                    nc.vector.tensor_copy(ot[:], pouts[cch][:])
                    nc.sync.dma_start(
                        outp[P * tt : P * (tt + 1), 512 * cch : 512 * (cch + 1)], ot[:]
                    )

    nc.compile()
    return nc


_CACHED_NC = None


def _get_program():
    global _CACHED_NC
    if _CACHED_NC is None:
        _CACHED_NC = _build_program()
    return _CACHED_NC


def _prepare_in_maps(x, W_qkv, b_qkv, W_proj):
    x = np.asarray(x, np.float32)
    W_qkv = np.asarray(W_qkv, np.float32)
    b_qkv = np.asarray(b_qkv, np.float32)
    W_proj = np.asarray(W_proj, np.float32)

    in_maps = []
    for core in range(NCORES):
        b, hg = core // 2, core % 2
        lo = hg * DL
        wq_s = W_qkv[:, lo : lo + DL]
        wk_s = W_qkv[:, C + lo : C + lo + DL]
        wv_s = W_qkv[:, 2 * C + lo : 2 * C + lo + DL]
        bq_s = b_qkv[lo : lo + DL].reshape(4, P).T
        bk_s = b_qkv[C + lo : C + lo + DL].reshape(4, P).T
        bv_s = np.broadcast_to(
            b_qkv[2 * C + lo : 2 * C + lo + DL].reshape(1, HPG, HD), (P, HPG, HD)
        )
        wp_s = W_proj[lo : lo + DL, :]
        if PV_BF16:
            wp_s = wp_s.astype(ml_dtypes.bfloat16)
        in_maps.append(
            {
                "xT": np.ascontiguousarray(x[b].T),
                "wq": np.ascontiguousarray(wq_s),
                "wk": np.ascontiguousarray(wk_s),
                "wv": np.ascontiguousarray(wv_s),
                "wp": np.ascontiguousarray(wp_s),
                "bq": np.ascontiguousarray(bq_s),
                "bk": np.ascontiguousarray(bk_s),
                "bv": np.ascontiguousarray(bv_s),
            }
        )
    return in_maps


def run(inputs, trace=False):
    nc = _get_program()
    in_maps = _prepare_in_maps(
        inputs["x"], inputs["W_qkv"], inputs["b_qkv"], inputs["W_proj"]
    )
    res = run_bass_kernel_spmd(nc, in_maps, core_ids=list(range(NCORES)), trace=trace)
    b_proj = np.asarray(inputs["b_proj"], np.float32)
    out = np.empty((B, T, C), np.float32)
    for b in range(B):
        out[b] = res.results[2 * b]["outp"] + res.results[2 * b + 1]["outp"] + b_proj
    return out, res


def kernel(**inputs):
    out, _ = run(inputs, trace=False)
    return out


# revision 11
# speedup vs baseline: 1.4956x; 1.4956x over previous
"""Causal self-attention (B=4, T=1024, C=1024, H=16) on 8 trn2 NeuronCores.

Sharding: core i handles batch b = i // 2 and head-group hg = i % 2
(8 heads = 512 of the 1024 channel dims). Each core computes

    qkv       = x[b] @ W_qkv[:, local]          (fp32r matmuls)
    P^T       = exp((k_h^T q_h) / 8) (causal)    (unstable softmax, bf16 P)
    y'^T      = [v_h | 1]^T @ P^T                (bf16, gives y^T + row-sums D)
    y^T       = y'^T / D                          (DMA remap + bcast + DVE)
    partial   = y^T.T @ W_proj[local, :]          (bf16)

Host sums the two head-group partials per batch and adds b_proj.

The qk projections, S^T matmuls, and V matmuls are software-pipelined per
head pair so the exp work on the scalar engine overlaps PE matmuls of the
next head pair. S^T blocks are column-restricted to the causal region and
diagonal blocks masked by a multiplicative upper-triangular mask post-exp.

Denominator normalization: the V matmul's ones-column gives row sums on
PSUM partition 64; a DMA remaps that row to SBUF partition 0 (gpsimd
partition_broadcast only reads physical partition 0), then broadcast +
reciprocal + multiply. Odd heads stage the normalized y at partition 0 and
DMA-remap into yT rows 64-127 (DVE ops never straddle partition bases).
"""

import numpy as np
from contextlib import ExitStack

import ml_dtypes

import concourse.bacc as bacc
import concourse.tile as tile
import concourse.mybir as mybir
from concourse.bass_utils import run_bass_kernel_spmd
from concourse.masks import make_upper_triangular

B, T, C, H, HD = 4, 1024, 1024, 16, 64
NCORES = 8
HPG = 8            # heads per core
DL = HPG * HD      # 512 local channel dims per core
P = 128

F32 = mybir.dt.float32
F32R = mybir.dt.float32r
BF16 = mybir.dt.bfloat16
EXP = mybir.ActivationFunctionType.Exp

MM_F32R = True     # fp32r for qkv / S^T matmuls (vs fp32, 4x slower)
PV_BF16 = True     # bf16 for P, v, y^T, wp (V-matmul + proj at full rate)

PV = BF16 if PV_BF16 else F32
MMDT = F32R if MM_F32R else F32


def _build_program():
    nc = bacc.Bacc("TRN2", target_bir_lowering=False)

    xT = nc.dram_tensor("xT", [C, T], MMDT, kind="ExternalInput").ap()
    wq = nc.dram_tensor("wq", [C, DL], MMDT, kind="ExternalInput").ap()
    wk = nc.dram_tensor("wk", [C, DL], MMDT, kind="ExternalInput").ap()
    wv = nc.dram_tensor("wv", [C, DL], MMDT, kind="ExternalInput").ap()
    wp = nc.dram_tensor("wp", [DL, C], PV, kind="ExternalInput").ap()
    bq = nc.dram_tensor("bq", [P, 4], F32, kind="ExternalInput").ap()
    bk = nc.dram_tensor("bk", [P, 4], F32, kind="ExternalInput").ap()
    bv = nc.dram_tensor("bv", [P, HPG, HD], F32, kind="ExternalInput").ap()
    outp = nc.dram_tensor("outp", [T, C], F32, kind="ExternalOutput").ap()

    with tile.TileContext(nc) as tc:
        with ExitStack() as ctx:
            consts = ctx.enter_context(tc.tile_pool(name="consts", bufs=1))
            xt_pool = ctx.enter_context(tc.tile_pool(name="xt", bufs=8))
            w_pool = ctx.enter_context(tc.tile_pool(name="w", bufs=4))
            qk_pool = ctx.enter_context(tc.tile_pool(name="qk", bufs=1))
            v_pool = ctx.enter_context(tc.tile_pool(name="v", bufs=8))
            pt_pool = ctx.enter_context(tc.tile_pool(name="pt", bufs=52))
            yt_pool = ctx.enter_context(tc.tile_pool(name="yt", bufs=1))
            d_pool = ctx.enter_context(tc.tile_pool(name="d", bufs=4))
            stg_pool = ctx.enter_context(tc.tile_pool(name="stg", bufs=3))
            out_pool = ctx.enter_context(tc.tile_pool(name="out", bufs=4))
            ps = ctx.enter_context(tc.tile_pool(name="ps", bufs=8, space="PSUM"))

            # ---- constants ----
            tri = consts.tile([P, P], PV, name="tri")  # 1 where tq >= s
            make_upper_triangular(nc, tri[:], val=1.0, diag=True)

            bq_sb = consts.tile([P, 4], F32, name="bq")
            nc.gpsimd.dma_start(bq_sb[:], bq)
            bk_sb = consts.tile([P, 4], F32, name="bk")
            nc.gpsimd.dma_start(bk_sb[:], bk)
            bv_sb = consts.tile([P, HPG, HD], F32, name="bv")
            nc.gpsimd.dma_start(bv_sb[:], bv)

            wp_sb = consts.tile([P, 4, C], PV, name="wp")  # loaded later

            xt = [xt_pool.tile([P, T], MMDT, name="xt") for _ in range(8)]

            # v tiles: [s=128, 8 heads x (64 dims + ones col)]
            v_sb = []
            for j in range(8):
                vt = v_pool.tile([P, HPG * (HD + 1)], PV, name="v")
                ones_cols = vt[:].rearrange("p (h e) -> p h e", e=HD + 1)[:, :, HD : HD + 1]
                nc.vector.memset(ones_cols, 1.0)
                v_sb.append(vt)

            qT_sb = qk_pool.tile([P, 4, T], MMDT, name="qT")
            kT_sb = qk_pool.tile([P, 4, T], MMDT, name="kT")
            yT_sb = yt_pool.tile([P, 4, T], PV, name="yT")

            # ---- v projection (needed by every head pair) ----
            ps_t = [ps.tile([P, 512], F32, name="ps") for _ in range(8)]
            for cc in range(8):
                nc.sync.dma_start(xt[cc][:], xT[P * cc : P * (cc + 1), :])
                wt = w_pool.tile([P, DL], MMDT, name="wv")
                nc.sync.dma_start(wt[:], wv[P * cc : P * (cc + 1), :])
                for tt in range(8):
                    nc.tensor.matmul(
                        ps_t[tt][:, :],
                        lhsT=xt[cc][:, P * tt : P * (tt + 1)],
                        rhs=wt[:],
                        start=(cc == 0),
                        stop=(cc == 7),
                    )
            for tt in range(8):
                out_ap = v_sb[tt][:].rearrange("p (h e) -> p h e", e=HD + 1)[:, :, 0:HD]
                in_ap = ps_t[tt][:].rearrange("p (h e) -> p h e", e=HD)
                nc.vector.tensor_add(out_ap, in_ap, bv_sb[:, :, :])

            def issue_qk(dt_):
                """q and k projections for head pair dt_ (128 channel dims)."""
                for w_dram, bias_sb, dest in ((wq, bq_sb, qT_sb), (wk, bk_sb, kT_sb)):
                    # one 3D DMA: [1024, 128] column slice -> [128, 8 cc, 128]
                    wt = w_pool.tile([P, 8, P], MMDT, name="wqk")
                    nc.sync.dma_start(
                        wt[:],
                        w_dram[:, P * dt_ : P * (dt_ + 1)].rearrange(
                            "(cc p) n -> p cc n", p=P
                        ),
                    )
                    pst = [ps.tile([P, 512], F32, name="ps") for _ in range(2)]
                    for cc in range(8):
                        for tch in range(2):
                            nc.tensor.matmul(
                                pst[tch][:, :],
                                lhsT=wt[:, cc, :],
                                rhs=xt[cc][:, 512 * tch : 512 * (tch + 1)],
                                start=(cc == 0),
                                stop=(cc == 7),
                            )
                    for tch in range(2):
                        nc.vector.tensor_scalar_add(
                            dest[:, dt_, 512 * tch : 512 * (tch + 1)],
                            pst[tch][:, :],
                            bias_sb[:, dt_ : dt_ + 1],
                        )

            def issue_st(hp):
                """S^T blocks + exp + causal mask for head pair hp."""
                pts = {}
                for c in range(2):
                    for j in range(4 * c + 4):
                        off = max(0, P * (j - 4 * c))
                        n = 512 - off
                        for half in range(2):
                            pr = 64 * half
                            pss = ps.tile([P, 512], F32, name="ps")
                            nc.tensor.matmul(
                                pss[:, :n],
                                lhsT=kT_sb[pr : pr + 64, hp, P * j : P * (j + 1)],
                                rhs=qT_sb[
                                    pr : pr + 64, hp, 512 * c + off : 512 * (c + 1)
                                ],
                                start=True,
                                stop=True,
                            )
                            pt = pt_pool.tile([P, 512], PV, name="pt")
                            nc.scalar.activation(
                                out=pt[:, off:512], in_=pss[:, :n], func=EXP, scale=0.125
                            )
                            if j >= 4 * c:
                                nc.vector.tensor_mul(
                                    pt[:, off : off + P], pt[:, off : off + P], tri[:]
                                )
                            pts[(half, c, j)] = pt
                return pts

            def issue_v(hp, pts):
                """V matmuls + denominator normalization for head pair hp."""
                for half in range(2):
                    h = 2 * hp + half
                    pr = 64 * half
                    for c in range(2):
                        jmax = 4 * c + 3
                        py = ps.tile([P, 512], F32, name="ps")
                        for j in range(jmax + 1):
                            off = max(0, P * (j - 4 * c))
                            nc.tensor.matmul(
                                py[0 : HD + 1, off:512],
                                lhsT=v_sb[j][:, (HD + 1) * h : (HD + 1) * (h + 1)],
                                rhs=pts[(half, c, j)][:, off:512],
                                start=(j == 0),
                                stop=(j == jmax),
                            )
                        ycp = stg_pool.tile([HD, 512], F32, name="ycp")
                        nc.vector.tensor_copy(ycp[:], py[0:HD, :])
                        d2 = d_pool.tile([P, 512], F32, name="d")
                        nc.vector.tensor_copy(d2[HD : HD + 1, :], py[HD : HD + 1, :])
                        nc.sync.dma_start(d2[0:1, :], d2[HD : HD + 1, :])
                        # custom DVE op only at partition base 0 (HW quirk)
                        nc.vector.reciprocal_approx_fast(d2[0:1, :], d2[0:1, :])
                        nc.gpsimd.partition_broadcast(d2[0:HD, :], d2[0:1, :])
                        dst = yT_sb[pr : pr + HD, hp, 512 * c : 512 * (c + 1)]
                        if half == 0:
                            nc.vector.tensor_mul(dst, ycp[:], d2[0:HD, :])
                        else:
                            stg = stg_pool.tile([HD, 512], PV, name="stg")
                            nc.vector.tensor_mul(stg[:], ycp[:], d2[0:HD, :])
                            nc.sync.dma_start(dst, stg[:])

            # ---- pipelined qk + attention ----
            issue_qk(0)
            pts_cur = issue_st(0)
            for hp in range(4):
                if hp + 1 < 4:
                    issue_qk(hp + 1)
                    pts_next = issue_st(hp + 1)
                else:
                    pts_next = None
                issue_v(hp, pts_cur)
                pts_cur = pts_next

            # ---- output projection ----
            for dc in range(4):
                nc.sync.dma_start(wp_sb[:, dc, :], wp[P * dc : P * (dc + 1), :])
            for tt in range(8):
                pouts = [ps.tile([P, 512], F32, name="ps") for _ in range(2)]
                for dc in range(4):
                    for cch in range(2):
                        nc.tensor.matmul(
                            pouts[cch][:, :],
                            lhsT=yT_sb[:, dc, P * tt : P * (tt + 1)],
                            rhs=wp_sb[:, dc, 512 * cch : 512 * (cch + 1)],
                            start=(dc == 0),
                            stop=(dc == 3),
                        )
                for cch in range(2):
                    ot = out_pool.tile([P, 512], F32, name="out")
                    nc.vector.tensor_copy(ot[:], pouts[cch][:])
                    nc.sync.dma_start(
                        outp[P * tt : P * (tt + 1), 512 * cch : 512 * (cch + 1)], ot[:]
                    )

    nc.compile()
    return nc


_CACHED_NC = None


def _get_program():
    global _CACHED_NC
    if _CACHED_NC is None:
        _CACHED_NC = _build_program()
    return _CACHED_NC


def _prepare_in_maps(x, W_qkv, b_qkv, W_proj):
    x = np.asarray(x, np.float32)
    W_qkv = np.asarray(W_qkv, np.float32)
    b_qkv = np.asarray(b_qkv, np.float32)
    W_proj = np.asarray(W_proj, np.float32)

    in_maps = []
    for core in range(NCORES):
        b, hg = core // 2, core % 2
        lo = hg * DL
        wq_s = W_qkv[:, lo : lo + DL]
        wk_s = W_qkv[:, C + lo : C + lo + DL]
        wv_s = W_qkv[:, 2 * C + lo : 2 * C + lo + DL]
        bq_s = b_qkv[lo : lo + DL].reshape(4, P).T
        bk_s = b_qkv[C + lo : C + lo + DL].reshape(4, P).T
        bv_s = np.broadcast_to(
            b_qkv[2 * C + lo : 2 * C + lo + DL].reshape(1, HPG, HD), (P, HPG, HD)
        )
        wp_s = W_proj[lo : lo + DL, :]
        if PV_BF16:
            wp_s = wp_s.astype(ml_dtypes.bfloat16)
        in_maps.append(
            {
                "xT": np.ascontiguousarray(x[b].T),
                "wq": np.ascontiguousarray(wq_s),
                "wk": np.ascontiguousarray(wk_s),
                "wv": np.ascontiguousarray(wv_s),
                "wp": np.ascontiguousarray(wp_s),
                "bq": np.ascontiguousarray(bq_s),
                "bk": np.ascontiguousarray(bk_s),
                "bv": np.ascontiguousarray(bv_s),
            }
        )
    return in_maps


def run(inputs, trace=False):
    nc = _get_program()
    in_maps = _prepare_in_maps(
        inputs["x"], inputs["W_qkv"], inputs["b_qkv"], inputs["W_proj"]
    )
    res = run_bass_kernel_spmd(nc, in_maps, core_ids=list(range(NCORES)), trace=trace)
    b_proj = np.asarray(inputs["b_proj"], np.float32)
    out = np.empty((B, T, C), np.float32)
    for b in range(B):
        out[b] = res.results[2 * b]["outp"] + res.results[2 * b + 1]["outp"] + b_proj
    return out, res


def kernel(**inputs):
    out, _ = run(inputs, trace=False)
    return out


# revision 12
# speedup vs baseline: 1.5277x; 1.0214x over previous
"""Causal self-attention (B=4, T=1024, C=1024, H=16) on 8 trn2 NeuronCores.

Sharding: core i handles batch b = i // 2 and head-group hg = i % 2
(8 heads = 512 of the 1024 channel dims). Each core computes

    qkv       = x[b] @ W_qkv[:, local]          (fp32r matmuls)
    P^T       = exp((k_h^T q_h) / 8) (causal)    (unstable softmax, bf16 P)
    y'^T      = [v_h | 1]^T @ P^T                (bf16, gives y^T + row-sums D)
    y^T       = y'^T / D                          (DMA remap + bcast + DVE)
    partial   = y^T.T @ W_proj[local, :]          (bf16)

Host sums the two head-group partials per batch and adds b_proj.

The qk projections, S^T matmuls, and V matmuls are software-pipelined per
head pair so the exp work on the scalar engine overlaps PE matmuls of the
next head pair. S^T blocks are column-restricted to the causal region and
diagonal blocks masked by a multiplicative upper-triangular mask post-exp.

Denominator normalization: the V matmul's ones-column gives row sums on
PSUM partition 64; a DMA remaps that row to SBUF partition 0 (gpsimd
partition_broadcast only reads physical partition 0), then broadcast +
reciprocal + multiply. Odd heads stage the normalized y at partition 0 and
DMA-remap into yT rows 64-127 (DVE ops never straddle partition bases).
"""

import numpy as np
from contextlib import ExitStack

import ml_dtypes

import concourse.bacc as bacc
import concourse.tile as tile
import concourse.mybir as mybir
from concourse.bass_utils import run_bass_kernel_spmd
from concourse.masks import make_upper_triangular

B, T, C, H, HD = 4, 1024, 1024, 16, 64
NCORES = 8
HPG = 8            # heads per core
DL = HPG * HD      # 512 local channel dims per core
P = 128

F32 = mybir.dt.float32
F32R = mybir.dt.float32r
BF16 = mybir.dt.bfloat16
EXP = mybir.ActivationFunctionType.Exp

MM_BF16 = True     # bf16 for qkv / S^T matmuls (FWL weight loads, 2x DMA)
PV_BF16 = True     # bf16 for P, v, y^T, wp (V-matmul + proj at full rate)

PV = BF16 if PV_BF16 else F32
MMDT = BF16 if MM_BF16 else F32R


def _build_program():
    nc = bacc.Bacc("TRN2", target_bir_lowering=False)

    xT = nc.dram_tensor("xT", [C, T], MMDT, kind="ExternalInput").ap()
    wq = nc.dram_tensor("wq", [C, DL], MMDT, kind="ExternalInput").ap()
    wk = nc.dram_tensor("wk", [C, DL], MMDT, kind="ExternalInput").ap()
    wv = nc.dram_tensor("wv", [C, DL], MMDT, kind="ExternalInput").ap()
    wp = nc.dram_tensor("wp", [DL, C], PV, kind="ExternalInput").ap()
    bq = nc.dram_tensor("bq", [P, 4], F32, kind="ExternalInput").ap()
    bk = nc.dram_tensor("bk", [P, 4], F32, kind="ExternalInput").ap()
    bv = nc.dram_tensor("bv", [P, HPG, HD], F32, kind="ExternalInput").ap()
    outp = nc.dram_tensor("outp", [T, C], F32, kind="ExternalOutput").ap()

    with tile.TileContext(nc) as tc:
        with ExitStack() as ctx:
            consts = ctx.enter_context(tc.tile_pool(name="consts", bufs=1))
            xt_pool = ctx.enter_context(tc.tile_pool(name="xt", bufs=8))
            w_pool = ctx.enter_context(tc.tile_pool(name="w", bufs=4))
            qk_pool = ctx.enter_context(tc.tile_pool(name="qk", bufs=1))
            v_pool = ctx.enter_context(tc.tile_pool(name="v", bufs=8))
            pt_pool = ctx.enter_context(tc.tile_pool(name="pt", bufs=52))
            yt_pool = ctx.enter_context(tc.tile_pool(name="yt", bufs=1))
            d_pool = ctx.enter_context(tc.tile_pool(name="d", bufs=4))
            stg_pool = ctx.enter_context(tc.tile_pool(name="stg", bufs=3))
            out_pool = ctx.enter_context(tc.tile_pool(name="out", bufs=4))
            ps = ctx.enter_context(tc.tile_pool(name="ps", bufs=8, space="PSUM"))

            # ---- constants ----
            tri = consts.tile([P, P], PV, name="tri")  # 1 where tq >= s
            make_upper_triangular(nc, tri[:], val=1.0, diag=True)

            bq_sb = consts.tile([P, 4], F32, name="bq")
            nc.gpsimd.dma_start(bq_sb[:], bq)
            bk_sb = consts.tile([P, 4], F32, name="bk")
            nc.gpsimd.dma_start(bk_sb[:], bk)
            bv_sb = consts.tile([P, HPG, HD], F32, name="bv")
            nc.gpsimd.dma_start(bv_sb[:], bv)

            wp_sb = consts.tile([P, 4, C], PV, name="wp")  # loaded later

            xt = [xt_pool.tile([P, T], MMDT, name="xt") for _ in range(8)]

            # v tiles: [s=128, 8 heads x (64 dims + ones col)]
            v_sb = []
            for j in range(8):
                vt = v_pool.tile([P, HPG * (HD + 1)], PV, name="v")
                ones_cols = vt[:].rearrange("p (h e) -> p h e", e=HD + 1)[:, :, HD : HD + 1]
                nc.vector.memset(ones_cols, 1.0)
                v_sb.append(vt)

            qT_sb = qk_pool.tile([P, 4, T], MMDT, name="qT")
            kT_sb = qk_pool.tile([P, 4, T], MMDT, name="kT")
            yT_sb = yt_pool.tile([P, 4, T], PV, name="yT")

            # ---- v projection (needed by every head pair) ----
            ps_t = [ps.tile([P, 512], F32, name="ps") for _ in range(8)]
            for cc in range(8):
                nc.sync.dma_start(xt[cc][:], xT[P * cc : P * (cc + 1), :])
                wt = w_pool.tile([P, DL], MMDT, name="wv")
                nc.sync.dma_start(wt[:], wv[P * cc : P * (cc + 1), :])
                for tt in range(8):
                    nc.tensor.matmul(
                        ps_t[tt][:, :],
                        lhsT=xt[cc][:, P * tt : P * (tt + 1)],
                        rhs=wt[:],
                        start=(cc == 0),
                        stop=(cc == 7),
                    )
            for tt in range(8):
                out_ap = v_sb[tt][:].rearrange("p (h e) -> p h e", e=HD + 1)[:, :, 0:HD]
                in_ap = ps_t[tt][:].rearrange("p (h e) -> p h e", e=HD)
                nc.vector.tensor_add(out_ap, in_ap, bv_sb[:, :, :])

            def issue_qk(dt_):
                """q and k projections for head pair dt_ (128 channel dims)."""
                for w_dram, bias_sb, dest in ((wq, bq_sb, qT_sb), (wk, bk_sb, kT_sb)):
                    # one 3D DMA: [1024, 128] column slice -> [128, 8 cc, 128]
                    wt = w_pool.tile([P, 8, P], MMDT, name="wqk")
                    nc.sync.dma_start(
                        wt[:],
                        w_dram[:, P * dt_ : P * (dt_ + 1)].rearrange(
                            "(cc p) n -> p cc n", p=P
                        ),
                    )
                    pst = [ps.tile([P, 512], F32, name="ps") for _ in range(2)]
                    for cc in range(8):
                        for tch in range(2):
                            nc.tensor.matmul(
                                pst[tch][:, :],
                                lhsT=wt[:, cc, :],
                                rhs=xt[cc][:, 512 * tch : 512 * (tch + 1)],
                                start=(cc == 0),
                                stop=(cc == 7),
                            )
                    for tch in range(2):
                        nc.vector.tensor_scalar_add(
                            dest[:, dt_, 512 * tch : 512 * (tch + 1)],
                            pst[tch][:, :],
                            bias_sb[:, dt_ : dt_ + 1],
                        )

            def issue_st(hp):
                """S^T blocks + exp + causal mask for head pair hp."""
                pts = {}
                for c in range(2):
                    for j in range(4 * c + 4):
                        off = max(0, P * (j - 4 * c))
                        n = 512 - off
                        for half in range(2):
                            pr = 64 * half
                            pss = ps.tile([P, 512], F32, name="ps")
                            nc.tensor.matmul(
                                pss[:, :n],
                                lhsT=kT_sb[pr : pr + 64, hp, P * j : P * (j + 1)],
                                rhs=qT_sb[
                                    pr : pr + 64, hp, 512 * c + off : 512 * (c + 1)
                                ],
                                start=True,
                                stop=True,
                            )
                            pt = pt_pool.tile([P, 512], PV, name="pt")
                            nc.scalar.activation(
                                out=pt[:, off:512], in_=pss[:, :n], func=EXP, scale=0.125
                            )
                            if j >= 4 * c:
                                nc.vector.tensor_mul(
                                    pt[:, off : off + P], pt[:, off : off + P], tri[:]
                                )
                            pts[(half, c, j)] = pt
                return pts

            def issue_v(hp, pts):
                """V matmuls + denominator normalization for head pair hp."""
                for half in range(2):
                    h = 2 * hp + half
                    pr = 64 * half
                    for c in range(2):
                        jmax = 4 * c + 3
                        py = ps.tile([P, 512], F32, name="ps")
                        for j in range(jmax + 1):
                            off = max(0, P * (j - 4 * c))
                            nc.tensor.matmul(
                                py[0 : HD + 1, off:512],
                                lhsT=v_sb[j][:, (HD + 1) * h : (HD + 1) * (h + 1)],
                                rhs=pts[(half, c, j)][:, off:512],
                                start=(j == 0),
                                stop=(j == jmax),
                            )
                        ycp = stg_pool.tile([HD, 512], F32, name="ycp")
                        nc.vector.tensor_copy(ycp[:], py[0:HD, :])
                        d2 = d_pool.tile([P, 512], F32, name="d")
                        nc.vector.tensor_copy(d2[HD : HD + 1, :], py[HD : HD + 1, :])
                        nc.sync.dma_start(d2[0:1, :], d2[HD : HD + 1, :])
                        # custom DVE op only at partition base 0 (HW quirk)
                        nc.vector.reciprocal_approx_fast(d2[0:1, :], d2[0:1, :])
                        nc.gpsimd.partition_broadcast(d2[0:HD, :], d2[0:1, :])
                        dst = yT_sb[pr : pr + HD, hp, 512 * c : 512 * (c + 1)]
                        if half == 0:
                            nc.vector.tensor_mul(dst, ycp[:], d2[0:HD, :])
                        else:
                            stg = stg_pool.tile([HD, 512], PV, name="stg")
                            nc.vector.tensor_mul(stg[:], ycp[:], d2[0:HD, :])
                            nc.sync.dma_start(dst, stg[:])

            # ---- pipelined qk + attention ----
            issue_qk(0)
            pts_cur = issue_st(0)
            for hp in range(4):
                if hp + 1 < 4:
                    issue_qk(hp + 1)
                    pts_next = issue_st(hp + 1)
                else:
                    pts_next = None
                issue_v(hp, pts_cur)
                pts_cur = pts_next

            # ---- output projection ----
            for dc in range(4):
                nc.sync.dma_start(wp_sb[:, dc, :], wp[P * dc : P * (dc + 1), :])
            for tt in range(8):
                pouts = [ps.tile([P, 512], F32, name="ps") for _ in range(2)]
                for dc in range(4):
                    for cch in range(2):
                        nc.tensor.matmul(
                            pouts[cch][:, :],
                            lhsT=yT_sb[:, dc, P * tt : P * (tt + 1)],
                            rhs=wp_sb[:, dc, 512 * cch : 512 * (cch + 1)],
                            start=(dc == 0),
                            stop=(dc == 3),
                        )
                for cch in range(2):
                    ot = out_pool.tile([P, 512], F32, name="out")
                    nc.vector.tensor_copy(ot[:], pouts[cch][:])
                    nc.sync.dma_start(
                        outp[P * tt : P * (tt + 1), 512 * cch : 512 * (cch + 1)], ot[:]
                    )

    nc.compile()
    return nc


_CACHED_NC = None


def _get_program():
    global _CACHED_NC
    if _CACHED_NC is None:
        _CACHED_NC = _build_program()
    return _CACHED_NC


def _prepare_in_maps(x, W_qkv, b_qkv, W_proj):
    x = np.asarray(x, np.float32)
    W_qkv = np.asarray(W_qkv, np.float32)
    b_qkv = np.asarray(b_qkv, np.float32)
    W_proj = np.asarray(W_proj, np.float32)

    in_maps = []
    for core in range(NCORES):
        b, hg = core // 2, core % 2
        lo = hg * DL
        wq_s = W_qkv[:, lo : lo + DL]
        wk_s = W_qkv[:, C + lo : C + lo + DL]
        wv_s = W_qkv[:, 2 * C + lo : 2 * C + lo + DL]
        bq_s = b_qkv[lo : lo + DL].reshape(4, P).T
        bk_s = b_qkv[C + lo : C + lo + DL].reshape(4, P).T
        bv_s = np.broadcast_to(
            b_qkv[2 * C + lo : 2 * C + lo + DL].reshape(1, HPG, HD), (P, HPG, HD)
        )
        wp_s = W_proj[lo : lo + DL, :]
        if PV_BF16:
            wp_s = wp_s.astype(ml_dtypes.bfloat16)
        mm_np = ml_dtypes.bfloat16 if MM_BF16 else np.float32
        in_maps.append(
            {
                "xT": np.ascontiguousarray(x[b].T).astype(mm_np),
                "wq": np.ascontiguousarray(wq_s).astype(mm_np),
                "wk": np.ascontiguousarray(wk_s).astype(mm_np),
                "wv": np.ascontiguousarray(wv_s).astype(mm_np),
                "wp": np.ascontiguousarray(wp_s),
                "bq": np.ascontiguousarray(bq_s),
                "bk": np.ascontiguousarray(bk_s),
                "bv": np.ascontiguousarray(bv_s),
            }
        )
    return in_maps


def run(inputs, trace=False):
    nc = _get_program()
    in_maps = _prepare_in_maps(
        inputs["x"], inputs["W_qkv"], inputs["b_qkv"], inputs["W_proj"]
    )
    res = run_bass_kernel_spmd(nc, in_maps, core_ids=list(range(NCORES)), trace=trace)
    b_proj = np.asarray(inputs["b_proj"], np.float32)
    out = np.empty((B, T, C), np.float32)
    for b in range(B):
        out[b] = res.results[2 * b]["outp"] + res.results[2 * b + 1]["outp"] + b_proj
    return out, res


def kernel(**inputs):
    out, _ = run(inputs, trace=False)
    return out
